# revision 10
# baseline (speedup 1.0000x reference)
"""Trainium2 Bass kernel for nn_Evaluate (nms_detection), v3.

Contract: kernel(**inputs) takes the FULL unsharded inputs
  pred_masks    [4, 256, 512, 512] f32
  target_masks  [4, 64, 512, 512]  f32
  pred_logits   [4, 256, 81]       f32
  target_clsIds [4, 64]            i32
and returns (precision, recall, accuracy) as float32 scalars, matching
reference.reference().

Sharding: 8 cores; core c handles batch b = c//2, pixel half h = c%2
(hw = 512*512 = 262144 pixels; halves of 131072).

v3 design (from v2's trace: Tensor 60.6us and Vector 54.6us busy at 77.6us
total -- the two engines were balanced with no slack):
- pred ships as 1 bit/(pixel,channel): 32 B/pixel, expanded to fp8 bytes
  (0x40 = 2.0) by 8 DVE shift+AND ops per tile (~2.6us/tile, was 3.9).
  Bit layout: packed byte m (of 32), bit j holds pred channel c = 32j + m,
  so shift j's output lands in contiguous SBUF columns [32j, 32j+32).
- tgt ships pre-expanded from HBM (0x40/0x00 bytes, 64 B/pixel) -- the DVE
  no longer touches it. DMA total 12.6 MB/core, ~25us on two queues.
- PE (mode="coltile"): per chunk one fp8 matmul K=128, M=64, N=256, with
  even/odd chunks issued to tile_position (0,0)/(0,64) -- two col-halves of
  the PE array run CONCURRENTLY (each holds its own tgt weights), writing
  acc[0:64,:] / acc[64:128,:] of one PSUM bank. Effective ~107ns per chunk
  pair vs DoubleRow's ~121ns + no DoubleRow MM penalty.
- mode="doublerow" keeps the v2 fp8 DoubleRow pair matmul as fallback.
acc[g or 64+g, p] = 4 * intersection (exact in f32: multiples of 4 < 2^21).
pred_sum / tgt_sum / tot_target come from host popcounts; the host
descales acc (exact) and runs the tiny greedy NMS + metrics in f32 math
mirroring the reference.
"""

import os
import sys
from contextlib import ExitStack

import numpy as np

for _p in ("/opt/trn_rl_repo", "/root/.axon_site/_ro/trn_rl_repo"):
    if os.path.isdir(_p) and _p not in sys.path:
        sys.path.insert(0, _p)

from concourse import bacc
import concourse.mybir as mybir
import concourse.tile as tile
from concourse.bass_utils import run_bass_kernel_spmd

BS = 4
P_CH = 256
G_CH = 64
NBP = P_CH // 8          # 32 packed pred bytes per pixel
HW_FULL = 512 * 512
N_CORES = 8
HW = HW_FULL // 2        # pixels per core
CHUNK = 128
T_CHUNKS = HW // CHUNK   # 1024 chunks per core
G = 64                   # chunks per tile
N_TILES = T_CHUNKS // G  # 16 tiles per core

SIZE_THRS = 1.0
CLS_SCORE_THR = 0.5
IOU_THR = 0.5

MODE = os.environ.get("KERNEL_MM_MODE", "coltile")

LAST_EXEC_TIME_NS = None
LAST_TRACE_PATH = None
LAST_ACC = None


def _install_ntff_hook():
    """Register the axon NTFF profiling hook that boot() skips when the
    image's antenv package lacks axon_hooks (see trn_agent_boot.trn_boot)."""
    import types

    try:
        import antenv
    except ImportError:
        return False
    if "antenv.axon_hooks" not in sys.modules:
        mod = types.ModuleType("antenv.axon_hooks")
        mod._hook = None

        def set_axon_ntff_profile_hook(h):
            mod._hook = h

        def get_axon_ntff_profile_hook():
            return mod._hook

        mod.set_axon_ntff_profile_hook = set_axon_ntff_profile_hook
        mod.get_axon_ntff_profile_hook = get_axon_ntff_profile_hook
        sys.modules["antenv.axon_hooks"] = mod
        antenv.axon_hooks = mod
    try:
        from antenv.axon_hooks import get_axon_ntff_profile_hook, set_axon_ntff_profile_hook

        if get_axon_ntff_profile_hook() is None:
            from trn_agent_boot.trn_boot import _ntff_profile_via_ctypes

            hook = _ntff_profile_via_ctypes("/opt/axon/libaxon_pjrt.so")
            if hook is None:
                return False
            set_axon_ntff_profile_hook(hook)
        return True
    except Exception:
        return False


def build_kernel(mode: str = MODE, nbuf: int = 8, nring: int = 3, n_warm: int = 22):
    nc = bacc.Bacc("TRN2", target_bir_lowering=False)

    pt = nc.dram_tensor("pt", [N_TILES, CHUNK, G, NBP], mybir.dt.uint8, kind="ExternalInput")
    gt = nc.dram_tensor("gt", [N_TILES, CHUNK, G, G_CH], mybir.dt.uint8, kind="ExternalInput")
    out = nc.dram_tensor("acc", [2 * G_CH, P_CH], mybir.dt.float32, kind="ExternalOutput")

    fp8 = mybir.dt.float8e4
    u16 = mybir.dt.uint16

    with ExitStack() as ctx:
        tc = ctx.enter_context(tile.TileContext(nc))
        nat_pool = ctx.enter_context(tc.tile_pool(name="nat", bufs=nbuf))
        gt_pool = ctx.enter_context(tc.tile_pool(name="gtp", bufs=nbuf))
        tb_pool = ctx.enter_context(tc.tile_pool(name="tb", bufs=nring))
        acc_pool = ctx.enter_context(tc.tile_pool(name="accp", bufs=1, space="PSUM"))
        misc_pool = ctx.enter_context(tc.tile_pool(name="misc", bufs=1))

        if mode == "coltile":
            # Two accumulation pairs (PSUM banks): chunks [0, T/2) -> acc0,
            # [T/2, T) -> acc1. acc0 is evacuated to SBUF by the (idle) scalar
            # engine WHILE acc1's matmuls run, and merged at the end with one
            # DVE tensor_tensor add -- takes the first copy off the tail.
            acc = [
                acc_pool.tile([2 * G_CH, P_CH], mybir.dt.float32, tag=f"acc{i}", name=f"acc{i}")
                for i in range(2)
            ]
        else:
            acc = [
                acc_pool.tile([G_CH, P_CH], mybir.dt.float32, tag=f"acc{i}", name=f"acc{i}")
                for i in range(2)
            ]

        # HAM warm-up: dummy full-array matmuls on a zeroed scratch tile into a
        # scratch PSUM bank, issued with no data deps so the PE is busy (and
        # transitions to K=8/8) while tile 0's DMA + expansion are in flight.
        if n_warm:
            warm_sb = misc_pool.tile([CHUNK, P_CH], fp8, tag="warm_sb")
            warm_ps = acc_pool.tile([CHUNK, 512], mybir.dt.float32, tag="warm_ps")
            eng = nc.gpsimd if hasattr(nc.gpsimd, "memset") else nc.vector
            eng.memset(warm_sb[:, :], 0.0)
            for _ in range(n_warm):
                nc.tensor.matmul(
                    warm_ps[:, 0:P_CH],
                    lhsT=warm_sb[:, 0:CHUNK],
                    rhs=warm_sb[:, :],
                    start=True,
                    stop=True,
                )

        gc = 0  # global chunk ordinal
        pair_idx = 0
        last_pair = T_CHUNKS // 2 - 1

        def expand(nat_t, tb_t, lo, hi):
            # Expand bit j of each packed byte into byte columns [32j, 32j+32):
            # out byte (2w+k) bit 6 <- in u16 word w bit (8k + j). 16-bit lanes
            # behave identically to 32-bit for this op (cross-byte bits are
            # masked) and qualify for the DVE 4x perf mode.
            for j in range(8):
                dst = tb_t[:, lo:hi, 32 * j : 32 * j + 32].bitcast(u16)
                src = nat_t[:, lo:hi, :].bitcast(u16)
                if j == 6:
                    nc.vector.tensor_scalar(
                        out=dst, in0=src,
                        scalar1=0x4040, scalar2=None,
                        op0=mybir.AluOpType.bitwise_and,
                    )
                elif j < 6:
                    nc.vector.tensor_scalar(
                        out=dst, in0=src,
                        scalar1=6 - j, scalar2=0x4040,
                        op0=mybir.AluOpType.logical_shift_left,
                        op1=mybir.AluOpType.bitwise_and,
                    )
                else:  # j == 7
                    nc.vector.tensor_scalar(
                        out=dst, in0=src,
                        scalar1=1, scalar2=0x4040,
                        op0=mybir.AluOpType.logical_shift_right,
                        op1=mybir.AluOpType.bitwise_and,
                    )

        HALF = T_CHUNKS // 2

        def matmuls(gtt_t, tb_t, lo, hi):
            nonlocal gc, pair_idx
            if mode == "coltile":
                for s in range(lo, hi):
                    col = (gc % 2) * G_CH
                    half = gc // HALF
                    rel = gc - half * HALF
                    nc.tensor.matmul(
                        acc[half][col : col + G_CH, :],
                        lhsT=gtt_t[:, s, :],
                        rhs=tb_t[:, s, :],
                        start=(rel < 2),
                        stop=(rel >= HALF - 2),
                        tile_position=(0, col),
                    )
                    gc += 1
            else:
                for s in range(lo, hi, 2):
                    nc.tensor.matmul(
                        acc[pair_idx % 2],
                        lhsT=gtt_t[:, s : s + 2, :],
                        rhs=tb_t[:, s : s + 2, :],
                        start=(pair_idx < 2),
                        stop=(pair_idx >= last_pair - 1),
                        perf_mode=mybir.MatmulPerfMode.DoubleRow,
                    )
                    pair_idx += 1

        SUB = 4          # tile 0 is split into SUB sub-blocks to cut startup
        SG = G // SUB
        half0_sb = misc_pool.tile([2 * G_CH, P_CH], mybir.dt.float32, tag="half0")
        for t in range(N_TILES):
            nat = nat_pool.tile([CHUNK, G, NBP], mybir.dt.uint8, tag="nat")
            gtt = gt_pool.tile([CHUNK, G, G_CH], fp8, tag="gtt")
            tb = tb_pool.tile([CHUNK, G, P_CH], fp8, tag="tb")
            gq = (nc.scalar, nc.gpsimd)[t % 2]  # split gt across two queues
            if t == 0:
                for u in range(SUB):
                    lo, hi = u * SG, (u + 1) * SG
                    nc.sync.dma_start(out=nat[:, lo:hi, :], in_=pt[t, :, lo:hi, :])
                    gq.dma_start(
                        out=gtt[:, lo:hi, :].bitcast(mybir.dt.uint8),
                        in_=gt[t, :, lo:hi, :])
                    expand(nat, tb, lo, hi)
                    matmuls(gtt, tb, lo, hi)
            else:
                nc.sync.dma_start(out=nat, in_=pt[t])
                gq.dma_start(out=gtt.bitcast(mybir.dt.uint8), in_=gt[t])
                expand(nat, tb, 0, G)
                matmuls(gtt, tb, 0, G)
            if mode == "coltile" and t == N_TILES // 2 - 1:
                # acc0 complete: evacuate on the idle scalar engine while the
                # second half's matmuls accumulate into acc1 (different bank).
                nc.scalar.activation(out=half0_sb, in_=acc[0],
                                     func=mybir.ActivationFunctionType.Copy)

        acc_sb = misc_pool.tile([2 * G_CH, P_CH], mybir.dt.float32)
        if mode == "coltile":
            nc.vector.tensor_tensor(out=acc_sb, in0=acc[1], in1=half0_sb,
                                    op=mybir.AluOpType.add)
        else:
            nc.vector.tensor_copy(out=acc_sb[0:G_CH, :], in_=acc[0])
            nc.scalar.activation(out=acc_sb[G_CH:, :], in_=acc[1],
                                 func=mybir.ActivationFunctionType.Copy)
        nc.sync.dma_start(out=out[0:G_CH, :], in_=acc_sb[0:G_CH, :])
        nc.scalar.dma_start(out=out[G_CH:, :], in_=acc_sb[G_CH:, :])

    nc.finalize()
    return nc


_NC_CACHE = None


def _get_nc():
    global _NC_CACHE
    if _NC_CACHE is None:
        _NC_CACHE = build_kernel()
    return _NC_CACHE


def _pack_inputs(pred_masks: np.ndarray, target_masks: np.ndarray):
    """pred: threshold + bit-pack 256 channels into 32 bytes/pixel (byte m
    bit j = channel 32j+m), per-core tiles [N_TILES, 128, G, 32] u8.
    tgt: expand to 0x40/0x00 bytes, per-core tiles [N_TILES, 128, G, 64] u8.
    Also returns exact pred/tgt popcount sums."""
    predb = (pred_masks.reshape(BS, P_CH, HW_FULL) > 0.5)
    tgtb = target_masks.reshape(BS, G_CH, HW_FULL) != 0
    pred_sum = predb.sum(axis=2, dtype=np.int64)          # [b, p]
    tgt_sum = tgtb.sum(axis=2, dtype=np.int64)            # [b, g]

    predu = predb.view(np.uint8)
    packed = np.zeros((BS, NBP, HW_FULL), np.uint8)       # [b, 32, hw]
    for j in range(8):
        np.bitwise_or(packed, predu[:, NBP * j : NBP * (j + 1), :] << j, out=packed)
    tgtx = tgtb.view(np.uint8) << 6                       # 0x40 / 0x00

    # per-core tiles: pixel = h*HW + (t*G + s)*128 + p -> [t, p, s, m/c]
    bigp = np.empty((BS, 2, N_TILES, CHUNK, G, NBP), np.uint8)
    bigt = np.empty((BS, 2, N_TILES, CHUNK, G, G_CH), np.uint8)
    for b in range(BS):
        for h in range(2):
            slab = packed[b, :, h * HW : (h + 1) * HW]    # [32, HW]
            v = slab.reshape(NBP, N_TILES, G, CHUNK)      # [m, t, s, p]
            bigp[b, h] = v.transpose(1, 3, 2, 0)          # [t, p, s, m]
            tsl = tgtx[b, :, h * HW : (h + 1) * HW]       # [64, HW]
            w = tsl.reshape(G_CH, N_TILES, G, CHUNK)      # [c, t, s, p]
            bigt[b, h] = w.transpose(1, 3, 2, 0)          # [t, p, s, c]
    return bigp, bigt, pred_sum, tgt_sum


def _run_device(pred_masks: np.ndarray, target_masks: np.ndarray):
    """Run the 8-core SPMD kernel; returns acc [BS, 65, 257] f64 legacy
    layout (intersection [g, p], pred_sum row 64, tgt_sum col 256)."""
    global LAST_EXEC_TIME_NS, LAST_TRACE_PATH, LAST_ACC
    nc = _get_nc()

    bigp, bigt, pred_sum, tgt_sum = _pack_inputs(pred_masks, target_masks)
    in_maps = []
    for c in range(N_CORES):
        b, h = divmod(c, 2)
        in_maps.append({"pt": bigp[b, h], "gt": bigt[b, h]})

    trace = bool(int(os.environ.get("KERNEL_TRACE", "0")))
    if trace:
        trace = _install_ntff_hook()
    kw = dict(trace=True) if trace else {}
    try:
        res = run_bass_kernel_spmd(nc, in_maps, core_ids=list(range(N_CORES)), **kw)
    except Exception:
        if not trace:
            raise
        res = run_bass_kernel_spmd(nc, in_maps, core_ids=list(range(N_CORES)))
    LAST_EXEC_TIME_NS = res.exec_time_ns
    if res.instructions_and_trace is not None:
        LAST_TRACE_PATH = res.instructions_and_trace[1]

    # Device: dev[g, p] + dev[64+g, p] = 4 * intersection (exact).
    acc = np.zeros((BS, G_CH + 1, P_CH + 1), np.float64)
    for c in range(N_CORES):
        b = c // 2
        dev = res.results[c]["acc"].astype(np.float64)
        acc[b, 0:G_CH, 0:P_CH] += (dev[0:G_CH] + dev[G_CH:]) / 4.0
    acc[:, G_CH, 0:P_CH] = pred_sum
    acc[:, 0:G_CH, P_CH] = tgt_sum
    LAST_ACC = acc
    return acc


def _greedy_match(iou, score, cls, psum, tcls):
    """Faithful numpy replica of reference._greedy_match (one batch)."""
    order = np.argsort(-score, kind="stable")
    iou_m = iou.copy()
    tp = 0.0
    fp = 0.0
    for pk in order:
        skip = (cls[pk] == 0) or (psum[pk] < SIZE_THRS) or (score[pk] < CLS_SCORE_THR)
        row = iou_m[pk]
        gk = int(np.argmax(row))
        hit = (row[gk] >= IOU_THR) and (cls[pk] == tcls[gk]) and (not skip)
        if hit:
            tp += 1.0
            iou_m[:, gk] = 0.0
        elif not skip:
            fp += 1.0
    return np.float32(tp), np.float32(fp)


def kernel(pred_masks, target_masks, pred_logits, target_clsIds):
    pred_masks = np.ascontiguousarray(np.asarray(pred_masks, dtype=np.float32))
    target_masks = np.ascontiguousarray(np.asarray(target_masks, dtype=np.float32))
    pred_logits = np.asarray(pred_logits, dtype=np.float32)
    target_clsIds = np.asarray(target_clsIds, dtype=np.int32)

    acc = _run_device(pred_masks, target_masks)

    # Host epilogue (tiny): iou + scores + greedy matching, all float32 math
    # mirroring the reference.
    intp = acc[:, 0:G_CH, 0:P_CH].transpose(0, 2, 1).astype(np.float32)  # [b, p, g]
    pred_sum = acc[:, G_CH, 0:P_CH].astype(np.float32)                   # [b, p]
    tgt_sum = acc[:, 0:G_CH, P_CH].astype(np.float32)                    # [b, g]

    union = pred_sum[:, :, None] + tgt_sum[:, None, :] - intp
    iou = intp / (union + np.float32(0.01))

    # softmax scores and argmax classes (fp32, same formula as jax.nn.softmax)
    m = pred_logits.max(axis=-1, keepdims=True)
    e = np.exp(pred_logits - m)
    sm = e / e.sum(axis=-1, keepdims=True)
    score = sm.max(axis=-1).astype(np.float32)                            # [b, p]
    cls = pred_logits.argmax(axis=-1).astype(np.int32)                    # [b, p]

    tp = np.float32(0.0)
    fp = np.float32(0.0)
    for b in range(BS):
        tp_b, fp_b = _greedy_match(iou[b], score[b], cls[b], pred_sum[b], target_clsIds[b])
        tp += tp_b
        fp += fp_b

    tot_target = np.float32((target_clsIds > 0).sum())
    precision = tp / (tp + fp + np.float32(0.001))
    recall = tp / (tot_target + np.float32(0.001))
    accuracy = tp / (tot_target + fp + np.float32(0.001))
    return (np.float32(precision), np.float32(recall), np.float32(accuracy))


# revision 11
# speedup vs baseline: 1.0889x; 1.0889x over previous
"""Trainium2 Bass kernel for nn_Evaluate (nms_detection), v3.

Contract: kernel(**inputs) takes the FULL unsharded inputs
  pred_masks    [4, 256, 512, 512] f32
  target_masks  [4, 64, 512, 512]  f32
  pred_logits   [4, 256, 81]       f32
  target_clsIds [4, 64]            i32
and returns (precision, recall, accuracy) as float32 scalars, matching
reference.reference().

Sharding: 8 cores; core c handles batch b = c//2, pixel half h = c%2
(hw = 512*512 = 262144 pixels; halves of 131072).

v3 design (from v2's trace: Tensor 60.6us and Vector 54.6us busy at 77.6us
total -- the two engines were balanced with no slack):
- pred ships as 1 bit/(pixel,channel): 32 B/pixel, expanded to fp8 bytes
  (0x40 = 2.0) by 8 DVE shift+AND ops per tile (~2.6us/tile, was 3.9).
  Bit layout: packed byte m (of 32), bit j holds pred channel c = 32j + m,
  so shift j's output lands in contiguous SBUF columns [32j, 32j+32).
- tgt ships pre-expanded from HBM (0x40/0x00 bytes, 64 B/pixel) -- the DVE
  no longer touches it. DMA total 12.6 MB/core, ~25us on two queues.
- PE (mode="coltile"): per chunk one fp8 matmul K=128, M=64, N=256, with
  even/odd chunks issued to tile_position (0,0)/(0,64) -- two col-halves of
  the PE array run CONCURRENTLY (each holds its own tgt weights), writing
  acc[0:64,:] / acc[64:128,:] of one PSUM bank. Effective ~107ns per chunk
  pair vs DoubleRow's ~121ns + no DoubleRow MM penalty.
- mode="doublerow" keeps the v2 fp8 DoubleRow pair matmul as fallback.
acc[g or 64+g, p] = 4 * intersection (exact in f32: multiples of 4 < 2^21).
pred_sum / tgt_sum / tot_target come from host popcounts; the host
descales acc (exact) and runs the tiny greedy NMS + metrics in f32 math
mirroring the reference.
"""

import os
import sys
from contextlib import ExitStack

import numpy as np

for _p in ("/opt/trn_rl_repo", "/root/.axon_site/_ro/trn_rl_repo"):
    if os.path.isdir(_p) and _p not in sys.path:
        sys.path.insert(0, _p)

from concourse import bacc
import concourse.mybir as mybir
import concourse.tile as tile
from concourse.bass_utils import run_bass_kernel_spmd

BS = 4
P_CH = 256
G_CH = 64
NBP = P_CH // 8          # 32 packed pred bytes per pixel
HW_FULL = 512 * 512
N_CORES = 8
HW = HW_FULL // 2        # pixels per core
CHUNK = 128
T_CHUNKS = HW // CHUNK   # 1024 chunks per core
G = 64                   # chunks per tile
N_TILES = T_CHUNKS // G  # 16 tiles per core

SIZE_THRS = 1.0
CLS_SCORE_THR = 0.5
IOU_THR = 0.5

MODE = os.environ.get("KERNEL_MM_MODE", "coltile")

LAST_EXEC_TIME_NS = None
LAST_TRACE_PATH = None
LAST_ACC = None


def _install_ntff_hook():
    """Register the axon NTFF profiling hook that boot() skips when the
    image's antenv package lacks axon_hooks (see trn_agent_boot.trn_boot)."""
    import types

    try:
        import antenv
    except ImportError:
        return False
    if "antenv.axon_hooks" not in sys.modules:
        mod = types.ModuleType("antenv.axon_hooks")
        mod._hook = None

        def set_axon_ntff_profile_hook(h):
            mod._hook = h

        def get_axon_ntff_profile_hook():
            return mod._hook

        mod.set_axon_ntff_profile_hook = set_axon_ntff_profile_hook
        mod.get_axon_ntff_profile_hook = get_axon_ntff_profile_hook
        sys.modules["antenv.axon_hooks"] = mod
        antenv.axon_hooks = mod
    try:
        from antenv.axon_hooks import get_axon_ntff_profile_hook, set_axon_ntff_profile_hook

        if get_axon_ntff_profile_hook() is None:
            from trn_agent_boot.trn_boot import _ntff_profile_via_ctypes

            hook = _ntff_profile_via_ctypes("/opt/axon/libaxon_pjrt.so")
            if hook is None:
                return False
            set_axon_ntff_profile_hook(hook)
        return True
    except Exception:
        return False


def build_kernel(mode: str = MODE, nbuf: int = 8, nring: int = 3, n_warm: int = 22):
    nc = bacc.Bacc("TRN2", target_bir_lowering=False)

    pt = nc.dram_tensor("pt", [N_TILES, CHUNK, G, NBP], mybir.dt.uint8, kind="ExternalInput")
    gt = nc.dram_tensor("gt", [N_TILES, CHUNK, G, G_CH], mybir.dt.uint8, kind="ExternalInput")
    out = nc.dram_tensor("acc", [2 * G_CH, P_CH], mybir.dt.float32, kind="ExternalOutput")

    fp8 = mybir.dt.float8e4
    u16 = mybir.dt.uint16

    with ExitStack() as ctx:
        tc = ctx.enter_context(tile.TileContext(nc))
        nat_pool = ctx.enter_context(tc.tile_pool(name="nat", bufs=nbuf))
        gt_pool = ctx.enter_context(tc.tile_pool(name="gtp", bufs=nbuf))
        tb_pool = ctx.enter_context(tc.tile_pool(name="tb", bufs=nring))
        acc_pool = ctx.enter_context(tc.tile_pool(name="accp", bufs=1, space="PSUM"))
        misc_pool = ctx.enter_context(tc.tile_pool(name="misc", bufs=1))

        if mode == "coltile":
            # Two accumulation pairs (PSUM banks): chunks [0, T/2) -> acc0,
            # [T/2, T) -> acc1. acc0 is evacuated to SBUF by the (idle) scalar
            # engine WHILE acc1's matmuls run, and merged at the end with one
            # DVE tensor_tensor add -- takes the first copy off the tail.
            acc = [
                acc_pool.tile([2 * G_CH, P_CH], mybir.dt.float32, tag=f"acc{i}", name=f"acc{i}")
                for i in range(2)
            ]
        else:
            acc = [
                acc_pool.tile([G_CH, P_CH], mybir.dt.float32, tag=f"acc{i}", name=f"acc{i}")
                for i in range(2)
            ]

        # HAM warm-up: dummy full-array matmuls on a zeroed scratch tile into a
        # scratch PSUM bank, issued with no data deps so the PE is busy (and
        # transitions to K=8/8) while tile 0's DMA + expansion are in flight.
        if n_warm:
            warm_sb = misc_pool.tile([CHUNK, P_CH], fp8, tag="warm_sb")
            warm_ps = acc_pool.tile([CHUNK, 512], mybir.dt.float32, tag="warm_ps")
            eng = nc.gpsimd if hasattr(nc.gpsimd, "memset") else nc.vector
            eng.memset(warm_sb[:, :], 0.0)
            for _ in range(n_warm):
                nc.tensor.matmul(
                    warm_ps[:, 0:P_CH],
                    lhsT=warm_sb[:, 0:CHUNK],
                    rhs=warm_sb[:, :],
                    start=True,
                    stop=True,
                )

        gc = 0  # global chunk ordinal
        pair_idx = 0
        last_pair = T_CHUNKS // 2 - 1

        def expand(nat_t, tb_t, lo, hi):
            # Expand bit j of each packed byte into byte columns [32j, 32j+32):
            # out byte (2w+k) bit 6 <- in u16 word w bit (8k + j). 16-bit lanes
            # behave identically to 32-bit for this op (cross-byte bits are
            # masked) and qualify for the DVE 4x perf mode.
            for j in range(8):
                dst = tb_t[:, lo:hi, 32 * j : 32 * j + 32].bitcast(u16)
                src = nat_t[:, lo:hi, :].bitcast(u16)
                if j == 6:
                    nc.vector.tensor_scalar(
                        out=dst, in0=src,
                        scalar1=0x4040, scalar2=None,
                        op0=mybir.AluOpType.bitwise_and,
                    )
                elif j < 6:
                    nc.vector.tensor_scalar(
                        out=dst, in0=src,
                        scalar1=6 - j, scalar2=0x4040,
                        op0=mybir.AluOpType.logical_shift_left,
                        op1=mybir.AluOpType.bitwise_and,
                    )
                else:  # j == 7
                    nc.vector.tensor_scalar(
                        out=dst, in0=src,
                        scalar1=1, scalar2=0x4040,
                        op0=mybir.AluOpType.logical_shift_right,
                        op1=mybir.AluOpType.bitwise_and,
                    )

        HALF = T_CHUNKS // 2

        def matmuls(gtt_t, tb_t, lo, hi):
            nonlocal gc, pair_idx
            if mode == "coltile":
                for s in range(lo, hi):
                    col = (gc % 2) * G_CH
                    half = gc // HALF
                    rel = gc - half * HALF
                    nc.tensor.matmul(
                        acc[half][col : col + G_CH, :],
                        lhsT=gtt_t[:, s, :],
                        rhs=tb_t[:, s, :],
                        start=(rel < 2),
                        stop=(rel >= HALF - 2),
                        tile_position=(0, col),
                    )
                    gc += 1
            else:
                for s in range(lo, hi, 2):
                    nc.tensor.matmul(
                        acc[pair_idx % 2],
                        lhsT=gtt_t[:, s : s + 2, :],
                        rhs=tb_t[:, s : s + 2, :],
                        start=(pair_idx < 2),
                        stop=(pair_idx >= last_pair - 1),
                        perf_mode=mybir.MatmulPerfMode.DoubleRow,
                    )
                    pair_idx += 1

        SUB = 4          # tile 0 is split into SUB sub-blocks to cut startup
        SG = G // SUB
        half0_sb = misc_pool.tile([2 * G_CH, P_CH], mybir.dt.float32, tag="half0")
        for t in range(N_TILES):
            nat = nat_pool.tile([CHUNK, G, NBP], mybir.dt.uint8, tag="nat")
            gtt = gt_pool.tile([CHUNK, G, G_CH], fp8, tag="gtt")
            tb = tb_pool.tile([CHUNK, G, P_CH], fp8, tag="tb")
            gq = nc.scalar  # gt queue (gpsimd's DMA queue measured ~2x slower)
            if t == 0:
                for u in range(SUB):
                    lo, hi = u * SG, (u + 1) * SG
                    nc.sync.dma_start(out=nat[:, lo:hi, :], in_=pt[t, :, lo:hi, :])
                    gq.dma_start(
                        out=gtt[:, lo:hi, :].bitcast(mybir.dt.uint8),
                        in_=gt[t, :, lo:hi, :])
                    expand(nat, tb, lo, hi)
                    matmuls(gtt, tb, lo, hi)
            else:
                nc.sync.dma_start(out=nat, in_=pt[t])
                gq.dma_start(out=gtt.bitcast(mybir.dt.uint8), in_=gt[t])
                expand(nat, tb, 0, G)
                matmuls(gtt, tb, 0, G)
            if mode == "coltile" and t == N_TILES // 2 - 1:
                # acc0 complete: evacuate on the idle scalar engine while the
                # second half's matmuls accumulate into acc1 (different bank).
                nc.scalar.activation(out=half0_sb, in_=acc[0],
                                     func=mybir.ActivationFunctionType.Copy)

        acc_sb = misc_pool.tile([2 * G_CH, P_CH], mybir.dt.float32)
        if mode == "coltile":
            nc.vector.tensor_tensor(out=acc_sb, in0=acc[1], in1=half0_sb,
                                    op=mybir.AluOpType.add)
        else:
            nc.vector.tensor_copy(out=acc_sb[0:G_CH, :], in_=acc[0])
            nc.scalar.activation(out=acc_sb[G_CH:, :], in_=acc[1],
                                 func=mybir.ActivationFunctionType.Copy)
        nc.sync.dma_start(out=out[0:G_CH, :], in_=acc_sb[0:G_CH, :])
        nc.scalar.dma_start(out=out[G_CH:, :], in_=acc_sb[G_CH:, :])

    nc.finalize()
    return nc


_NC_CACHE = None


def _get_nc():
    global _NC_CACHE
    if _NC_CACHE is None:
        _NC_CACHE = build_kernel()
    return _NC_CACHE


def _pack_inputs(pred_masks: np.ndarray, target_masks: np.ndarray):
    """pred: threshold + bit-pack 256 channels into 32 bytes/pixel (byte m
    bit j = channel 32j+m), per-core tiles [N_TILES, 128, G, 32] u8.
    tgt: expand to 0x40/0x00 bytes, per-core tiles [N_TILES, 128, G, 64] u8.
    Also returns exact pred/tgt popcount sums."""
    predb = (pred_masks.reshape(BS, P_CH, HW_FULL) > 0.5)
    tgtb = target_masks.reshape(BS, G_CH, HW_FULL) != 0
    pred_sum = predb.sum(axis=2, dtype=np.int64)          # [b, p]
    tgt_sum = tgtb.sum(axis=2, dtype=np.int64)            # [b, g]

    predu = predb.view(np.uint8)
    packed = np.zeros((BS, NBP, HW_FULL), np.uint8)       # [b, 32, hw]
    for j in range(8):
        np.bitwise_or(packed, predu[:, NBP * j : NBP * (j + 1), :] << j, out=packed)
    tgtx = tgtb.view(np.uint8) << 6                       # 0x40 / 0x00

    # per-core tiles: pixel = h*HW + (t*G + s)*128 + p -> [t, p, s, m/c]
    bigp = np.empty((BS, 2, N_TILES, CHUNK, G, NBP), np.uint8)
    bigt = np.empty((BS, 2, N_TILES, CHUNK, G, G_CH), np.uint8)
    for b in range(BS):
        for h in range(2):
            slab = packed[b, :, h * HW : (h + 1) * HW]    # [32, HW]
            v = slab.reshape(NBP, N_TILES, G, CHUNK)      # [m, t, s, p]
            bigp[b, h] = v.transpose(1, 3, 2, 0)          # [t, p, s, m]
            tsl = tgtx[b, :, h * HW : (h + 1) * HW]       # [64, HW]
            w = tsl.reshape(G_CH, N_TILES, G, CHUNK)      # [c, t, s, p]
            bigt[b, h] = w.transpose(1, 3, 2, 0)          # [t, p, s, c]
    return bigp, bigt, pred_sum, tgt_sum


def _run_device(pred_masks: np.ndarray, target_masks: np.ndarray):
    """Run the 8-core SPMD kernel; returns acc [BS, 65, 257] f64 legacy
    layout (intersection [g, p], pred_sum row 64, tgt_sum col 256)."""
    global LAST_EXEC_TIME_NS, LAST_TRACE_PATH, LAST_ACC
    nc = _get_nc()

    bigp, bigt, pred_sum, tgt_sum = _pack_inputs(pred_masks, target_masks)
    in_maps = []
    for c in range(N_CORES):
        b, h = divmod(c, 2)
        in_maps.append({"pt": bigp[b, h], "gt": bigt[b, h]})

    trace = bool(int(os.environ.get("KERNEL_TRACE", "0")))
    if trace:
        trace = _install_ntff_hook()
    kw = dict(trace=True) if trace else {}
    try:
        res = run_bass_kernel_spmd(nc, in_maps, core_ids=list(range(N_CORES)), **kw)
    except Exception:
        if not trace:
            raise
        res = run_bass_kernel_spmd(nc, in_maps, core_ids=list(range(N_CORES)))
    LAST_EXEC_TIME_NS = res.exec_time_ns
    if res.instructions_and_trace is not None:
        LAST_TRACE_PATH = res.instructions_and_trace[1]

    # Device: dev[g, p] + dev[64+g, p] = 4 * intersection (exact).
    acc = np.zeros((BS, G_CH + 1, P_CH + 1), np.float64)
    for c in range(N_CORES):
        b = c // 2
        dev = res.results[c]["acc"].astype(np.float64)
        acc[b, 0:G_CH, 0:P_CH] += (dev[0:G_CH] + dev[G_CH:]) / 4.0
    acc[:, G_CH, 0:P_CH] = pred_sum
    acc[:, 0:G_CH, P_CH] = tgt_sum
    LAST_ACC = acc
    return acc


def _greedy_match(iou, score, cls, psum, tcls):
    """Faithful numpy replica of reference._greedy_match (one batch)."""
    order = np.argsort(-score, kind="stable")
    iou_m = iou.copy()
    tp = 0.0
    fp = 0.0
    for pk in order:
        skip = (cls[pk] == 0) or (psum[pk] < SIZE_THRS) or (score[pk] < CLS_SCORE_THR)
        row = iou_m[pk]
        gk = int(np.argmax(row))
        hit = (row[gk] >= IOU_THR) and (cls[pk] == tcls[gk]) and (not skip)
        if hit:
            tp += 1.0
            iou_m[:, gk] = 0.0
        elif not skip:
            fp += 1.0
    return np.float32(tp), np.float32(fp)


def kernel(pred_masks, target_masks, pred_logits, target_clsIds):
    pred_masks = np.ascontiguousarray(np.asarray(pred_masks, dtype=np.float32))
    target_masks = np.ascontiguousarray(np.asarray(target_masks, dtype=np.float32))
    pred_logits = np.asarray(pred_logits, dtype=np.float32)
    target_clsIds = np.asarray(target_clsIds, dtype=np.int32)

    acc = _run_device(pred_masks, target_masks)

    # Host epilogue (tiny): iou + scores + greedy matching, all float32 math
    # mirroring the reference.
    intp = acc[:, 0:G_CH, 0:P_CH].transpose(0, 2, 1).astype(np.float32)  # [b, p, g]
    pred_sum = acc[:, G_CH, 0:P_CH].astype(np.float32)                   # [b, p]
    tgt_sum = acc[:, 0:G_CH, P_CH].astype(np.float32)                    # [b, g]

    union = pred_sum[:, :, None] + tgt_sum[:, None, :] - intp
    iou = intp / (union + np.float32(0.01))

    # softmax scores and argmax classes (fp32, same formula as jax.nn.softmax)
    m = pred_logits.max(axis=-1, keepdims=True)
    e = np.exp(pred_logits - m)
    sm = e / e.sum(axis=-1, keepdims=True)
    score = sm.max(axis=-1).astype(np.float32)                            # [b, p]
    cls = pred_logits.argmax(axis=-1).astype(np.int32)                    # [b, p]

    tp = np.float32(0.0)
    fp = np.float32(0.0)
    for b in range(BS):
        tp_b, fp_b = _greedy_match(iou[b], score[b], cls[b], pred_sum[b], target_clsIds[b])
        tp += tp_b
        fp += fp_b

    tot_target = np.float32((target_clsIds > 0).sum())
    precision = tp / (tp + fp + np.float32(0.001))
    recall = tp / (tot_target + np.float32(0.001))
    accuracy = tp / (tot_target + fp + np.float32(0.001))
    return (np.float32(precision), np.float32(recall), np.float32(accuracy))
